# revision 58
# baseline (speedup 1.0000x reference)
"""Causal multi-head attention (RoPE) on 8 Trainium2 NeuronCores.

Sharding: (batch=2) x (head groups=4) -> 8 cores. Core c = 4*b + g handles
batch b, heads [4g, 4g+4); it computes QKV projections for its 256
projection rows, full causal attention for its 4 heads, and its partial
o_proj contribution y_part[b]; the host sums the 4 partials per batch.

Precision strategy (tolerance 2e-2; measured end-to-end ~1.3e-2 on HW):
  - fp16 for all matmul operands (same PE rate as f32r/bf16, half the DMA)
  - fp8e4 ONLY for rotated Q/K: S = K^T Q runs as one fp8 DoubleRow matmul
    per 128-key block (contraction 32 partitions x 2 k-tiles = 64 head
    dims), halving the biggest attention matmul at ~1.3e-2 total error.
    wq/wk carry a x4 host-side scale (fewer fp8 subnormals), folded into
    the exp scale. All other fp8 stages measured over budget; they stay
    fp16 (full-fp8 variant measured 5e-2 -- quantization noise is NOT
    attenuated by softmax averaging, the signal shrinks identically).

Per-core kernel (one Tile program; engines balanced PE~100us, ACT~79,
DVE~64, Pool~50; exp is ACT-only since GPSIMD has no PSUM port and DVE
has no exp):
  - QKV projections: fp16 matmuls, contraction d=1024 in 8 tiles. Q/K
    emerge in an [even-dims | odd-dims] row permutation so RoPE needs no
    partition swaps; rope = 4 muls (DVE, fp16 2x mode) + 2 adds (Pool;
    DVE for chunk 0's critical path) against fp16 cos/sin tables, written
    as fp8 into qt/kt[128 = 4 heads x 32 freqs, 2 (even|odd), seq].
  - S PSUM groups of 2 key blocks [128, 1024], trimmed to the causal band.
  - softmax exp on ACT (no max-subtraction; |scores| < ~5), fp16 out; the
    causal triangle is zeroed AFTERWARD on the fp16 weights by Pool
    affine_select (scores are finite so post-exp zeroing is exact).
  - AV: fp16 matmuls per key block, query range trimmed at the diagonal;
    ones-column appended to V gives the denominator row for a deferred
    1/l normalization (recip DVE + broadcast Pool + mul DVE).
  - o_proj: fp16 matmuls; y streamed out per 128x512 block in fp16.

Schedule: attention (S -> exp -> zero -> AV with AV deferred av_lag
groups, software-pipelined across heads) is the spine; projections of
chunk c+1 and o_proj of chunk c-2 are sliced into fine-grained "filler"
steps pulled between attention groups so the PE queue never lumps, with
chunk 3's V-projection and two o_proj chunks reserved for the ACT-bound
last chunk.
"""

import sys

for _p in ("/opt/trn_rl_repo",):
    if _p not in sys.path:
        sys.path.insert(0, _p)

import numpy as np

SEQ = 2048
D_MODEL = 1024
NUM_HEADS = 16
HEAD_DIM = 64
THETA = 10000.0
N_CORES = 8
HEADS_PER_CORE = 4
KSLICE = HEADS_PER_CORE * HEAD_DIM  # 256 projection rows per core
CH = 512  # query-chunk width


def build_nc(seq=SEQ, proj_copy_eng="dve", v_eng="dve", ys_split=0, mul_eng="dve",
             add_eng="pool", tri_eng="pool", av_lag=6):
    import concourse.mybir as mybir
    import concourse.tile as tile
    from concourse import bacc
    from contextlib import ExitStack

    f32 = mybir.dt.float32
    f16 = mybir.dt.float16
    f8 = mybir.dt.float8e4
    Exp = mybir.ActivationFunctionType.Exp
    DR = mybir.MatmulPerfMode.DoubleRow

    nd = D_MODEL // 128          # 8 x 128 contraction tiles
    nch = seq // CH              # 4 query chunks
    nib = seq // 128             # 16 key blocks
    VS = 68                      # per-head stride in vt blocks

    nc = bacc.Bacc(None, target_bir_lowering=False)

    xT = nc.declare_dram_parameter("xT", [D_MODEL, seq], f16, isOutput=False)
    wq = nc.declare_dram_parameter("wq", [D_MODEL, KSLICE], f16, isOutput=False)
    wk = nc.declare_dram_parameter("wk", [D_MODEL, KSLICE], f16, isOutput=False)
    wv = nc.declare_dram_parameter("wv", [D_MODEL, KSLICE], f16, isOutput=False)
    wo = nc.declare_dram_parameter("wo", [KSLICE, D_MODEL], f16, isOutput=False)
    cosI = nc.declare_dram_parameter("cosI", [128, seq], f16, isOutput=False)
    sinI = nc.declare_dram_parameter("sinI", [128, seq], f16, isOutput=False)
    y = nc.declare_dram_parameter("y", [seq, D_MODEL], f16, isOutput=True)

    with tile.TileContext(nc) as tc, ExitStack() as ctx:
        persist = ctx.enter_context(tc.tile_pool(name="persist", bufs=1))

        qt = persist.tile([128, 2, seq], f8, tag="qt")
        kt = persist.tile([128, 2, seq], f8, tag="kt")
        vt = persist.tile([128, nib, 4 * VS], f16, tag="vt")
        outTn = [persist.tile([128, seq], f16, tag=f"outTn{p}", name=f"outTn{p}")
                 for p in range(2)]
        wq_s = persist.tile([128, nd, KSLICE], f16, tag="wq_s")
        wk_s = persist.tile([128, nd, KSLICE], f16, tag="wk_s")
        wv_s = persist.tile([128, nd, KSLICE], f16, tag="wv_s")
        wo_s = persist.tile([128, 2, D_MODEL], f16, tag="wo_s")
        cos_s = persist.tile([128, seq], f16, tag="cos_s")
        sin_s = persist.tile([128, seq], f16, tag="sin_s")

        # ones column per head in V_aug (col VS*h+64)
        for h in range(HEADS_PER_CORE):
            nc.gpsimd.memset(vt[:, :, VS * h + 64:VS * h + 65], 1.0)

        with (
            tc.tile_pool(name="p1x", bufs=2) as p1x,
            tc.tile_pool(name="rtmp", bufs=8) as rtmp,
            tc.tile_pool(name="pts", bufs=6) as ptsp,
            tc.tile_pool(name="ysp", bufs=4) as ysp,
            tc.tile_pool(name="lip", bufs=4) as lip,
            tc.tile_pool(name="lbp", bufs=4) as lbp,
            tc.tile_pool(name="ppool", bufs=2, space="PSUM") as ppool,
            tc.tile_pool(name="sgp", bufs=2, space="PSUM") as sgp,
            tc.tile_pool(name="pop", bufs=2, space="PSUM") as pop,
        ):
            xr = xT[:].rearrange("(t p) s -> p t s", p=128)
            xts = {}

            def emit_loads(c):
                if c >= nch:
                    return
                xt = p1x.tile([128, nd, CH], f16, tag="xt")
                nc.sync.dma_start(out=xt[:], in_=xr[:, :, c * CH:(c + 1) * CH])
                xts[c] = xt

            # upfront loads: 4-dt halves so proj(0) starts after two DMAs
            # while the HWDGE (625ns dispatch each) keeps pace; wk and the
            # chunk-0 table slices come early (chunk-0 rope critical path)
            wqr = wq[:].rearrange("(t p) k -> p t k", p=128)
            wkr = wk[:].rearrange("(t p) k -> p t k", p=128)
            xt0 = p1x.tile([128, nd, CH], f16, tag="xt", name="xt0")
            for quarter in range(4):
                hs = slice(2 * quarter, 2 * quarter + 2)
                nc.sync.dma_start(out=wq_s[:, hs, :], in_=wqr[:, hs, :])
                nc.sync.dma_start(out=xt0[:, hs, :], in_=xr[:, hs, 0:CH])
            xts[0] = xt0
            nc.sync.dma_start(out=wk_s[:], in_=wkr)
            nc.sync.dma_start(out=cos_s[:, 0:CH], in_=cosI[:, 0:CH])
            nc.sync.dma_start(out=sin_s[:, 0:CH], in_=sinI[:, 0:CH])
            nc.sync.dma_start(out=wv_s[:], in_=wv[:].rearrange("(t p) k -> p t k", p=128))
            nc.sync.dma_start(out=cos_s[:, CH:seq], in_=cosI[:, CH:seq])
            nc.sync.dma_start(out=sin_s[:, CH:seq], in_=sinI[:, CH:seq])
            emit_loads(1)
            nc.sync.dma_start(out=wo_s[:], in_=wo[:].rearrange("(q p) d -> p q d", p=128))

            def copy_psum(out, in_, eng):
                if eng == "act":
                    nc.scalar.copy(out=out, in_=in_)
                else:
                    nc.vector.tensor_copy(out=out, in_=in_)

            def proj_qk_steps(c):
                """Generator of fine-grained emission steps for Q/K proj+rope."""
                csl = slice(c * CH, (c + 1) * CH)
                xt = xts[c]
                cos_c, sin_c = cos_s[:, csl], sin_s[:, csl]
                for w_s, dst in ((wq_s, qt), (wk_s, kt)):
                    pb = []
                    for eo in range(2):
                        pp = ppool.tile([128, CH], f32, tag="pp", name="pp")
                        for t in range(nd):
                            nc.tensor.matmul(
                                pp[:],
                                lhsT=w_s[:, t, 128 * eo:128 * eo + 128],
                                rhs=xt[:, t, :],
                                start=(t == 0), stop=(t == nd - 1),
                            )
                            if t % 4 == 3:
                                yield
                        pbt = rtmp.tile([128, CH], f16, tag="pb")
                        copy_psum(pbt[:], pp[:], proj_copy_eng)
                        pb.append(pbt)
                        yield
                    # rope: r_e = e*cos - o*sin ; r_o = e*sin + o*cos
                    prods = []
                    for (src, tab) in ((pb[0], cos_c), (pb[1], sin_c),
                                       (pb[0], sin_c), (pb[1], cos_c)):
                        pr = rtmp.tile([128, CH], f16, tag="pr")
                        if mul_eng == "dve":
                            nc.vector.tensor_mul(pr[:], src[:], tab)
                        else:
                            nc.gpsimd.tensor_mul(pr[:], src[:], tab)
                        prods.append(pr)
                    if add_eng == "pool" and c > 0:
                        nc.gpsimd.tensor_sub(dst[:, 0, csl], prods[0][:], prods[1][:])
                        nc.gpsimd.tensor_add(dst[:, 1, csl], prods[2][:], prods[3][:])
                    else:
                        nc.vector.tensor_sub(dst[:, 0, csl], prods[0][:], prods[1][:])
                        nc.vector.tensor_add(dst[:, 1, csl], prods[2][:], prods[3][:])
                    yield

            def proj_v_steps(c, ibls=None, pop_xt=True):
                xt = xts.pop(c) if pop_xt else xts[c]
                for ibl in (range(CH // 128) if ibls is None else ibls):
                    ib = c * (CH // 128) + ibl
                    vp = ppool.tile([128, KSLICE], f32, tag="pp", name="vp")
                    for t in range(nd):
                        nc.tensor.matmul(
                            vp[:],
                            lhsT=xt[:, t, ibl * 128:ibl * 128 + 128],
                            rhs=wv_s[:, t, :],
                            start=(t == 0), stop=(t == nd - 1),
                        )
                        if t % 4 == 3:
                            yield
                    copy_psum(
                        vt[:, ib, :].rearrange("p (h z) -> p h z", z=VS)[:, :, 0:64],
                        vp[:].rearrange("p (h z) -> p h z", z=64),
                        v_eng,
                    )
                    yield

            def oproj_steps(c, ys_eng=None):
                for ib in range(c * (CH // 128), (c + 1) * (CH // 128)):
                    for ns in range(2):
                        yp = ppool.tile([128, 512], f32, tag="pp", name="yp")
                        for p2 in range(2):
                            nc.tensor.matmul(
                                yp[:],
                                lhsT=outTn[p2][:, ib * 128:ib * 128 + 128],
                                rhs=wo_s[:, p2, ns * 512:ns * 512 + 512],
                                start=(p2 == 0), stop=(p2 == 1),
                            )
                        ys = ysp.tile([128, 512], f16, tag="ys")
                        eng = ys_eng
                        if eng == "alt":
                            eng = "act" if (2 * ib + ns) % 2 == 0 else "dve"
                        elif eng is None:
                            eng = "act" if (ys_split > 0 and (ib + ns) % ys_split == 0) else "dve"
                        last = (ib == nib - 1 and ns == 1)
                        if last:
                            # final store: halves on both engines + two DMAs
                            # to shorten the end-of-kernel drain chain
                            nc.scalar.copy(out=ys[:, 0:256], in_=yp[:, 0:256])
                            nc.vector.tensor_copy(out=ys[:, 256:512], in_=yp[:, 256:512])
                            for hl in range(2):
                                nc.sync.dma_start(
                                    out=y[ib * 128:ib * 128 + 128,
                                          ns * 512 + 256 * hl:ns * 512 + 256 * hl + 256],
                                    in_=ys[:, 256 * hl:256 * hl + 256],
                                )
                        else:
                            copy_psum(ys[:], yp[:], eng)
                            nc.sync.dma_start(
                                out=y[ib * 128:ib * 128 + 128, ns * 512:ns * 512 + 512],
                                in_=ys[:],
                            )
                        yield

            def drain(gen):
                for _ in gen:
                    pass

            drain(proj_qk_steps(0))
            drain(proj_v_steps(0))

            import itertools
            _SENTINEL = object()

            for c in range(nch):
                csl = slice(c * CH, (c + 1) * CH)
                i_lo = c * CH
                ngr = 2 * (c + 1)
                njb = 4 * (c + 1)
                # filler plan: shift o_proj a chunk later and split chunk 3's
                # V projection so the ACT-bound last chunk has PE work
                fillers, n_steps, front = [], 0, 1.0
                if c + 1 < nch:
                    fillers.append(proj_qk_steps(c + 1))
                    n_steps += 14
                    if c + 1 == nch - 1:
                        fillers.append(proj_v_steps(c + 1, ibls=(0, 1), pop_xt=False))
                        n_steps += 6
                    else:
                        fillers.append(proj_v_steps(c + 1))
                        n_steps += 12
                if c == nch - 1:
                    fillers.append(proj_v_steps(c, ibls=(2, 3)))
                    n_steps += 6
                    front = 1.6
                if c == nch - 1:
                    for cc in range(nch - 1):
                        fillers.append(oproj_steps(cc))
                        n_steps += 8
                filler = itertools.chain(*fillers)
                n_groups = HEADS_PER_CORE * ngr
                done_steps = [0]

                def pull(frac):
                    want = int(round(min(1.0, frac * front) * n_steps))
                    while done_steps[0] < want:
                        if next(filler, _SENTINEL) is _SENTINEL:
                            done_steps[0] = n_steps
                            return
                        done_steps[0] += 1

                def normalize(h, po_t):
                    # out = po[0:64] * (1/l) -> fp16.  For the final head of
                    # the final chunk, split into 128-col pieces so the first
                    # o_proj block starts ~1.5us earlier.
                    nsplit = 4 if (c == nch - 1 and h == HEADS_PER_CORE - 1) else 1
                    hb = 64 * (h % 2)
                    w = CH // nsplit
                    for p_ in range(nsplit):
                        ps = slice(p_ * w, (p_ + 1) * w)
                        li = lip.tile([1, w], f32, tag="li")
                        nc.vector.reciprocal(out=li[:], in_=po_t[64:65, ps])
                        lb = lbp.tile([64, w], f32, tag="lb")
                        nc.gpsimd.partition_broadcast(lb[:], li[:])
                        nc.vector.tensor_mul(
                            outTn[h // 2][hb:hb + 64, c * CH + p_ * w:
                                          c * CH + (p_ + 1) * w],
                            po_t[0:64, ps], lb[:])

                # software-pipelined over all (head, group) pairs of the
                # chunk: S/exp run one group ahead of AV so head boundaries
                # don't stall the exp stream
                order = [(h, g) for h in range(HEADS_PER_CORE) for g in range(ngr)]
                po_tiles = {}
                pend = {}

                def emit_av(h, g):
                    pts_t, offs = pend.pop((h, g))
                    po_t = po_tiles[h]
                    for s2 in range(2):
                        jb = 2 * g + s2
                        off = offs[s2]
                        nc.tensor.matmul(
                            po_t[:, off:CH],
                            lhsT=vt[:, jb, VS * h:VS * h + 65],
                            rhs=pts_t[:, s2, off:CH],
                            start=(jb == 0), stop=(jb == njb - 1),
                        )
                    if g == ngr - 1:
                        normalize(h, po_tiles.pop(h))
                        if h == 0:
                            emit_loads(c + 2)

                for idx, (h, g) in enumerate(order):
                    hp = slice(32 * h, 32 * h + 32)
                    if g == 0:
                        po_tiles[h] = pop.tile([65, CH], f32, tag="po", name="po")
                    sg_t = sgp.tile([128, 2, CH], f32, tag="sg")
                    offs = []
                    for s2 in range(2):
                        jb = 2 * g + s2
                        off = max(0, jb * 128 - i_lo)
                        offs.append(off)
                        nc.tensor.matmul(
                            sg_t[:, s2, off:CH],
                            lhsT=kt[hp, :, jb * 128:jb * 128 + 128],
                            rhs=qt[hp, :, i_lo + off:i_lo + CH],
                            start=True, stop=True,
                            perf_mode=DR,
                            tile_position=(32 * h, 0),
                        )
                    # wq/wk carry a x4 host-side scale (fp8 subnormal margin);
                    # scores are x16, folded into the exp scale
                    pts_t = ptsp.tile([128, 2, CH], f16, tag="pt")
                    if offs[0] >= 256:
                        # narrow diagonal pair: trim exp to the valid cols
                        for s2 in range(2):
                            nc.scalar.activation(
                                out=pts_t[:, s2, offs[s2]:CH],
                                in_=sg_t[:, s2, offs[s2]:CH],
                                func=Exp, scale=0.125 / 16)
                    else:
                        nc.scalar.activation(
                            out=pts_t[:], in_=sg_t[:], func=Exp, scale=0.125 / 16)
                    # causal triangle -> 0 on the fp16 weights (post-exp)
                    for s2 in range(2):
                        off = (2 * g + s2) * 128 - i_lo
                        if off >= 0:
                            region = pts_t[:, s2, off:off + 128]
                            nc.gpsimd.affine_select(
                                out=region, in_=region,
                                compare_op=mybir.AluOpType.is_ge,
                                fill=0.0, base=0,
                                pattern=[[1, 128]], channel_multiplier=-1,
                            )
                    pend[(h, g)] = (pts_t, offs)
                    # drain the AV pipeline toward the end of the last chunk
                    # so the tail isn't a serial run of deferred AVs
                    lag = av_lag
                    if c == nch - 1 and idx >= n_groups - 4:
                        lag = 2
                    if idx >= lag:
                        for j in range(idx - av_lag, idx - lag + 1):
                            if j >= 0 and order[j] in pend:
                                emit_av(*order[j])
                    pull((idx + 1) / n_groups)
                for k in range(len(order)):
                    if order[k] in pend:
                        emit_av(*order[k])
                pull(1.0)

            drain(oproj_steps(nch - 1, ys_eng="alt"))

    nc.finalize()
    return nc


def make_in_maps(in_features, q_proj, k_proj, v_proj, o_proj, token_positions, seq=SEQ):
    """Host-side sharding: core c = 4*b + g handles batch b, heads [4g, 4g+4)."""
    f16 = np.float16

    x = np.asarray(in_features, np.float32)
    pos = np.asarray(token_positions, np.float64)
    half = HEAD_DIM // 2
    inv = 1.0 / (THETA ** (2.0 * np.arange(half) / HEAD_DIM))
    ang = pos[:, None] * inv[None, :]           # [seq, 32]
    cosT = np.cos(ang).T                        # [32, seq]
    sinT = np.sin(ang).T
    cosI = np.ascontiguousarray(np.tile(cosT, (4, 1))).astype(f16)
    sinI = np.ascontiguousarray(np.tile(sinT, (4, 1))).astype(f16)

    qp = np.asarray(q_proj, np.float32)
    kp = np.asarray(k_proj, np.float32)
    vp = np.asarray(v_proj, np.float32)
    op = np.asarray(o_proj, np.float32)

    xTb = [np.ascontiguousarray(x[b].T).astype(f16) for b in range(x.shape[0])]
    in_maps = []
    for core in range(N_CORES):
        b, g = divmod(core, HEADS_PER_CORE)
        # row permutation: evens block then odds block, 4 heads x 32
        pe = np.concatenate([64 * (4 * g + h) + np.arange(0, HEAD_DIM, 2)
                             for h in range(HEADS_PER_CORE)])
        po = np.concatenate([64 * (4 * g + h) + np.arange(1, HEAD_DIM, 2)
                             for h in range(HEADS_PER_CORE)])
        perm = np.concatenate([pe, po])
        ks = slice(g * KSLICE, (g + 1) * KSLICE)
        in_maps.append({
            "xT": xTb[b],
            "wq": np.ascontiguousarray(qp[perm].T * 4.0).astype(f16),
            "wk": np.ascontiguousarray(kp[perm].T * 4.0).astype(f16),
            "wv": np.ascontiguousarray(vp[ks].T).astype(f16),
            "wo": np.ascontiguousarray(op[:, ks].T).astype(f16),
            "cosI": cosI,
            "sinI": sinI,
        })
    return in_maps


def assemble(results, batch=2):
    ys = []
    for b in range(batch):
        parts = [results[b * HEADS_PER_CORE + g]["y"].astype(np.float64)
                 for g in range(HEADS_PER_CORE)]
        ys.append(np.sum(parts, axis=0, dtype=np.float64).astype(np.float32))
    return np.stack(ys, axis=0)


_NC_CACHE = {}


def get_nc(seq=SEQ):
    if seq not in _NC_CACHE:
        _NC_CACHE[seq] = build_nc(seq)
    return _NC_CACHE[seq]


def kernel(**inputs):
    from concourse.bass_utils import run_bass_kernel_spmd

    nc = get_nc()
    in_maps = make_in_maps(**inputs)
    res = run_bass_kernel_spmd(nc, in_maps, list(range(N_CORES)))
    return assemble(res.results)


if __name__ == "__main__":
    rng = np.random.default_rng(0)
    ins = {
        "in_features": rng.standard_normal((2, SEQ, D_MODEL), np.float32),
        "q_proj": (rng.standard_normal((D_MODEL, D_MODEL)) * 0.02).astype(np.float32),
        "k_proj": (rng.standard_normal((D_MODEL, D_MODEL)) * 0.02).astype(np.float32),
        "v_proj": (rng.standard_normal((D_MODEL, D_MODEL)) * 0.02).astype(np.float32),
        "o_proj": (rng.standard_normal((D_MODEL, D_MODEL)) * 0.02).astype(np.float32),
        "token_positions": np.arange(SEQ, dtype=np.int32),
    }
    out = kernel(**ins)
    print("kernel output:", out.shape, out.dtype)


# revision 70
# speedup vs baseline: 1.0038x; 1.0038x over previous
"""Causal multi-head attention (RoPE) on 8 Trainium2 NeuronCores.

Sharding: (batch=2) x (head groups=4) -> 8 cores. Core c = 4*b + g handles
batch b, heads [4g, 4g+4); it computes QKV projections for its 256
projection rows, full causal attention for its 4 heads, and its partial
o_proj contribution y_part[b]; the host sums the 4 partials per batch.

Precision strategy (tolerance 2e-2; measured end-to-end ~1.3e-2 on HW):
  - fp16 for all matmul operands (same PE rate as f32r/bf16, half the DMA)
  - fp8e4 ONLY for rotated Q/K: S = K^T Q runs as one fp8 DoubleRow matmul
    per 128-key block (contraction 32 partitions x 2 k-tiles = 64 head
    dims), halving the biggest attention matmul at ~1.3e-2 total error.
    wq/wk carry a x4 host-side scale (fewer fp8 subnormals), folded into
    the exp scale. All other fp8 stages measured over budget; they stay
    fp16 (full-fp8 variant measured 5e-2 -- quantization noise is NOT
    attenuated by softmax averaging, the signal shrinks identically).

Per-core kernel (one Tile program; engines balanced PE~100us, ACT~79,
DVE~64, Pool~50; exp is ACT-only since GPSIMD has no PSUM port and DVE
has no exp):
  - QKV projections: fp16 matmuls, contraction d=1024 in 8 tiles. Q/K
    emerge in an [even-dims | odd-dims] row permutation so RoPE needs no
    partition swaps; rope = 4 muls (DVE, fp16 2x mode) + 2 adds (Pool;
    DVE for chunk 0's critical path) against fp16 cos/sin tables, written
    as fp8 into qt/kt[128 = 4 heads x 32 freqs, 2 (even|odd), seq].
  - S PSUM groups of 2 key blocks [128, 1024], trimmed to the causal band.
  - softmax exp on ACT (no max-subtraction; |scores| < ~5), fp16 out; the
    causal triangle is zeroed AFTERWARD on the fp16 weights by Pool
    affine_select (scores are finite so post-exp zeroing is exact).
  - AV: fp16 matmuls per key block, query range trimmed at the diagonal;
    ones-column appended to V gives the denominator row for a deferred
    1/l normalization (recip DVE + broadcast Pool + mul DVE).
  - o_proj: fp16 matmuls; y streamed out per 128x512 block in fp16.

Schedule: attention (S -> exp -> zero -> AV with AV deferred av_lag
groups, software-pipelined across heads) is the spine; projections of
chunk c+1 and o_proj of chunk c-2 are sliced into fine-grained "filler"
steps pulled between attention groups so the PE queue never lumps, with
chunk 3's V-projection and two o_proj chunks reserved for the ACT-bound
last chunk.
"""

import sys

for _p in ("/opt/trn_rl_repo",):
    if _p not in sys.path:
        sys.path.insert(0, _p)

import numpy as np

SEQ = 2048
D_MODEL = 1024
NUM_HEADS = 16
HEAD_DIM = 64
THETA = 10000.0
N_CORES = 8
HEADS_PER_CORE = 4
KSLICE = HEADS_PER_CORE * HEAD_DIM  # 256 projection rows per core
CH = 512  # query-chunk width


def build_nc(seq=SEQ, proj_copy_eng="dve", v_eng="dve", ys_split=0, mul_eng="dve",
             add_eng="pool", tri_eng="pool", av_lag=6):
    import concourse.mybir as mybir
    import concourse.tile as tile
    from concourse import bacc
    from contextlib import ExitStack

    f32 = mybir.dt.float32
    f16 = mybir.dt.float16
    f8 = mybir.dt.float8e4
    Exp = mybir.ActivationFunctionType.Exp
    DR = mybir.MatmulPerfMode.DoubleRow

    nd = D_MODEL // 128          # 8 x 128 contraction tiles
    nch = seq // CH              # 4 query chunks
    nib = seq // 128             # 16 key blocks
    VS = 68                      # per-head stride in vt blocks

    nc = bacc.Bacc(None, target_bir_lowering=False)

    xT = nc.declare_dram_parameter("xT", [D_MODEL, seq], f16, isOutput=False)
    wq = nc.declare_dram_parameter("wq", [D_MODEL, KSLICE], f16, isOutput=False)
    wk = nc.declare_dram_parameter("wk", [D_MODEL, KSLICE], f16, isOutput=False)
    wv = nc.declare_dram_parameter("wv", [D_MODEL, KSLICE], f16, isOutput=False)
    wo = nc.declare_dram_parameter("wo", [KSLICE, D_MODEL], f16, isOutput=False)
    cosI = nc.declare_dram_parameter("cosI", [128, seq], f16, isOutput=False)
    sinI = nc.declare_dram_parameter("sinI", [128, seq], f16, isOutput=False)
    y = nc.declare_dram_parameter("y", [seq, D_MODEL], f16, isOutput=True)

    with tile.TileContext(nc) as tc, ExitStack() as ctx:
        persist = ctx.enter_context(tc.tile_pool(name="persist", bufs=1))

        qt = persist.tile([128, 2, seq], f8, tag="qt")
        kt = persist.tile([128, 2, seq], f8, tag="kt")
        vt = persist.tile([128, nib, 4 * VS], f16, tag="vt")
        outTn = [persist.tile([128, seq], f16, tag=f"outTn{p}", name=f"outTn{p}")
                 for p in range(2)]
        wq_s = persist.tile([128, nd, KSLICE], f16, tag="wq_s")
        wk_s = persist.tile([128, nd, KSLICE], f16, tag="wk_s")
        wv_s = persist.tile([128, nd, KSLICE], f16, tag="wv_s")
        wo_s = persist.tile([128, 2, D_MODEL], f16, tag="wo_s")
        cos_s = persist.tile([128, seq], f16, tag="cos_s")
        sin_s = persist.tile([128, seq], f16, tag="sin_s")

        # ones column per head in V_aug (col VS*h+64)
        for h in range(HEADS_PER_CORE):
            nc.gpsimd.memset(vt[:, :, VS * h + 64:VS * h + 65], 1.0)

        with (
            tc.tile_pool(name="p1x", bufs=3) as p1x,
            tc.tile_pool(name="rtmp", bufs=10) as rtmp,
            tc.tile_pool(name="pts", bufs=7) as ptsp,
            tc.tile_pool(name="ysp", bufs=8) as ysp,
            tc.tile_pool(name="lip", bufs=4) as lip,
            tc.tile_pool(name="lbp", bufs=4) as lbp,
            tc.tile_pool(name="ppool", bufs=2, space="PSUM") as ppool,
            tc.tile_pool(name="sgp", bufs=2, space="PSUM") as sgp,
            tc.tile_pool(name="pop", bufs=2, space="PSUM") as pop,
        ):
            xr = xT[:].rearrange("(t p) s -> p t s", p=128)
            xts = {}

            def emit_loads(c):
                if c >= nch:
                    return
                xt = p1x.tile([128, nd, CH], f16, tag="xt")
                nc.sync.dma_start(out=xt[:], in_=xr[:, :, c * CH:(c + 1) * CH])
                xts[c] = xt

            # upfront loads: 4-dt halves so proj(0) starts after two DMAs
            # while the HWDGE (625ns dispatch each) keeps pace; wk and the
            # chunk-0 table slices come early (chunk-0 rope critical path)
            wqr = wq[:].rearrange("(t p) k -> p t k", p=128)
            wkr = wk[:].rearrange("(t p) k -> p t k", p=128)
            xt0 = p1x.tile([128, nd, CH], f16, tag="xt", name="xt0")
            for quarter in range(4):
                hs = slice(2 * quarter, 2 * quarter + 2)
                nc.sync.dma_start(out=wq_s[:, hs, :], in_=wqr[:, hs, :])
                nc.sync.dma_start(out=xt0[:, hs, :], in_=xr[:, hs, 0:CH])
            xts[0] = xt0
            nc.sync.dma_start(out=wk_s[:], in_=wkr)
            nc.sync.dma_start(out=cos_s[:, 0:CH], in_=cosI[:, 0:CH])
            nc.sync.dma_start(out=sin_s[:, 0:CH], in_=sinI[:, 0:CH])
            nc.sync.dma_start(out=wv_s[:], in_=wv[:].rearrange("(t p) k -> p t k", p=128))
            nc.sync.dma_start(out=cos_s[:, CH:seq], in_=cosI[:, CH:seq])
            nc.sync.dma_start(out=sin_s[:, CH:seq], in_=sinI[:, CH:seq])
            emit_loads(1)
            nc.sync.dma_start(out=wo_s[:], in_=wo[:].rearrange("(q p) d -> p q d", p=128))

            def copy_psum(out, in_, eng):
                if eng == "act":
                    nc.scalar.copy(out=out, in_=in_)
                else:
                    nc.vector.tensor_copy(out=out, in_=in_)

            def proj_qk_steps(c):
                """Generator of fine-grained emission steps for Q/K proj+rope."""
                csl = slice(c * CH, (c + 1) * CH)
                xt = xts[c]
                cos_c, sin_c = cos_s[:, csl], sin_s[:, csl]
                for w_s, dst in ((wq_s, qt), (wk_s, kt)):
                    pb = []
                    for eo in range(2):
                        pp = ppool.tile([128, CH], f32, tag="pp", name="pp")
                        for t in range(nd):
                            nc.tensor.matmul(
                                pp[:],
                                lhsT=w_s[:, t, 128 * eo:128 * eo + 128],
                                rhs=xt[:, t, :],
                                start=(t == 0), stop=(t == nd - 1),
                            )
                            if t % 4 == 3:
                                yield
                        pbt = rtmp.tile([128, CH], f16, tag="pb")
                        copy_psum(pbt[:], pp[:], proj_copy_eng)
                        pb.append(pbt)
                        yield
                    # rope: r_e = e*cos - o*sin ; r_o = e*sin + o*cos
                    prods = []
                    for (src, tab) in ((pb[0], cos_c), (pb[1], sin_c),
                                       (pb[0], sin_c), (pb[1], cos_c)):
                        pr = rtmp.tile([128, CH], f16, tag="pr")
                        if mul_eng == "dve":
                            nc.vector.tensor_mul(pr[:], src[:], tab)
                        else:
                            nc.gpsimd.tensor_mul(pr[:], src[:], tab)
                        prods.append(pr)
                    if add_eng == "pool" and c > 0:
                        nc.gpsimd.tensor_sub(dst[:, 0, csl], prods[0][:], prods[1][:])
                        nc.gpsimd.tensor_add(dst[:, 1, csl], prods[2][:], prods[3][:])
                    else:
                        nc.vector.tensor_sub(dst[:, 0, csl], prods[0][:], prods[1][:])
                        nc.vector.tensor_add(dst[:, 1, csl], prods[2][:], prods[3][:])
                    yield

            def proj_v_steps(c, ibls=None, pop_xt=True):
                xt = xts.pop(c) if pop_xt else xts[c]
                for ibl in (range(CH // 128) if ibls is None else ibls):
                    ib = c * (CH // 128) + ibl
                    vp = ppool.tile([128, KSLICE], f32, tag="pp", name="vp")
                    for t in range(nd):
                        nc.tensor.matmul(
                            vp[:],
                            lhsT=xt[:, t, ibl * 128:ibl * 128 + 128],
                            rhs=wv_s[:, t, :],
                            start=(t == 0), stop=(t == nd - 1),
                        )
                        if t % 4 == 3:
                            yield
                    copy_psum(
                        vt[:, ib, :].rearrange("p (h z) -> p h z", z=VS)[:, :, 0:64],
                        vp[:].rearrange("p (h z) -> p h z", z=64),
                        v_eng,
                    )
                    yield

            def oproj_steps(c, ys_eng=None):
                for ib in range(c * (CH // 128), (c + 1) * (CH // 128)):
                    for ns in range(2):
                        yp = ppool.tile([128, 512], f32, tag="pp", name="yp")
                        for p2 in range(2):
                            nc.tensor.matmul(
                                yp[:],
                                lhsT=outTn[p2][:, ib * 128:ib * 128 + 128],
                                rhs=wo_s[:, p2, ns * 512:ns * 512 + 512],
                                start=(p2 == 0), stop=(p2 == 1),
                            )
                        ys = ysp.tile([128, 512], f16, tag="ys")
                        eng = ys_eng
                        if eng == "alt":
                            eng = "act" if (2 * ib + ns) % 2 == 0 else "dve"
                        elif eng is None:
                            eng = "act" if (ys_split > 0 and (ib + ns) % ys_split == 0) else "dve"
                        last = (ib == nib - 1 and ns == 1)
                        if last:
                            # final store: halves on both engines + two DMAs
                            # to shorten the end-of-kernel drain chain
                            nc.scalar.copy(out=ys[:, 0:256], in_=yp[:, 0:256])
                            nc.vector.tensor_copy(out=ys[:, 256:512], in_=yp[:, 256:512])
                            for hl in range(2):
                                nc.sync.dma_start(
                                    out=y[ib * 128:ib * 128 + 128,
                                          ns * 512 + 256 * hl:ns * 512 + 256 * hl + 256],
                                    in_=ys[:, 256 * hl:256 * hl + 256],
                                )
                        else:
                            copy_psum(ys[:], yp[:], eng)
                            nc.sync.dma_start(
                                out=y[ib * 128:ib * 128 + 128, ns * 512:ns * 512 + 512],
                                in_=ys[:],
                            )
                        yield

            def drain(gen):
                for _ in gen:
                    pass

            drain(proj_qk_steps(0))
            drain(proj_v_steps(0))

            import itertools
            _SENTINEL = object()

            for c in range(nch):
                csl = slice(c * CH, (c + 1) * CH)
                i_lo = c * CH
                ngr = 2 * (c + 1)
                njb = 4 * (c + 1)
                # filler plan: shift o_proj a chunk later and split chunk 3's
                # V projection so the ACT-bound last chunk has PE work
                fillers, n_steps, front = [], 0, 1.0
                if c + 1 < nch:
                    fillers.append(proj_qk_steps(c + 1))
                    n_steps += 14
                    if c + 1 == nch - 1:
                        fillers.append(proj_v_steps(c + 1, ibls=(0, 1), pop_xt=False))
                        n_steps += 6
                    else:
                        fillers.append(proj_v_steps(c + 1))
                        n_steps += 12
                if c == nch - 1:
                    fillers.append(proj_v_steps(c, ibls=(2, 3)))
                    n_steps += 6
                    front = 1.6
                if c == nch - 1:
                    for cc in range(nch - 1):
                        fillers.append(oproj_steps(cc))
                        n_steps += 8
                filler = itertools.chain(*fillers)
                n_groups = HEADS_PER_CORE * ngr
                done_steps = [0]

                def pull(frac):
                    want = int(round(min(1.0, frac * front) * n_steps))
                    while done_steps[0] < want:
                        if next(filler, _SENTINEL) is _SENTINEL:
                            done_steps[0] = n_steps
                            return
                        done_steps[0] += 1

                def normalize(h, po_t):
                    # out = po[0:64] * (1/l) -> fp16.  For the final head of
                    # the final chunk, split into 128-col pieces so the first
                    # o_proj block starts ~1.5us earlier.
                    nsplit = 4 if (c == nch - 1 and h == HEADS_PER_CORE - 1) else 1
                    hb = 64 * (h % 2)
                    w = CH // nsplit
                    for p_ in range(nsplit):
                        ps = slice(p_ * w, (p_ + 1) * w)
                        li = lip.tile([1, w], f32, tag="li")
                        nc.vector.reciprocal(out=li[:], in_=po_t[64:65, ps])
                        lb = lbp.tile([64, w], f32, tag="lb")
                        nc.gpsimd.partition_broadcast(lb[:], li[:])
                        nc.vector.tensor_mul(
                            outTn[h // 2][hb:hb + 64, c * CH + p_ * w:
                                          c * CH + (p_ + 1) * w],
                            po_t[0:64, ps], lb[:])

                # software-pipelined over all (head, group) pairs of the
                # chunk: S/exp run one group ahead of AV so head boundaries
                # don't stall the exp stream
                order = [(h, g) for h in range(HEADS_PER_CORE) for g in range(ngr)]
                po_tiles = {}
                pend = {}

                def emit_av(h, g):
                    pts_t, offs = pend.pop((h, g))
                    po_t = po_tiles[h]
                    for s2 in range(2):
                        jb = 2 * g + s2
                        off = offs[s2]
                        nc.tensor.matmul(
                            po_t[:, off:CH],
                            lhsT=vt[:, jb, VS * h:VS * h + 65],
                            rhs=pts_t[:, s2, off:CH],
                            start=(jb == 0), stop=(jb == njb - 1),
                        )
                    if g == ngr - 1:
                        normalize(h, po_tiles.pop(h))
                        if h == 0:
                            emit_loads(c + 2)

                for idx, (h, g) in enumerate(order):
                    hp = slice(32 * h, 32 * h + 32)
                    if g == 0:
                        po_tiles[h] = pop.tile([65, CH], f32, tag="po", name="po")
                    sg_t = sgp.tile([128, 2, CH], f32, tag="sg")
                    offs = []
                    for s2 in range(2):
                        jb = 2 * g + s2
                        off = max(0, jb * 128 - i_lo)
                        offs.append(off)
                        nc.tensor.matmul(
                            sg_t[:, s2, off:CH],
                            lhsT=kt[hp, :, jb * 128:jb * 128 + 128],
                            rhs=qt[hp, :, i_lo + off:i_lo + CH],
                            start=True, stop=True,
                            perf_mode=DR,
                            tile_position=(32 * h, 0),
                        )
                    # wq/wk carry a x4 host-side scale (fp8 subnormal margin);
                    # scores are x16, folded into the exp scale
                    pts_t = ptsp.tile([128, 2, CH], f16, tag="pt")
                    if offs[0] >= 256:
                        # narrow diagonal pair: trim exp to the valid cols
                        for s2 in range(2):
                            nc.scalar.activation(
                                out=pts_t[:, s2, offs[s2]:CH],
                                in_=sg_t[:, s2, offs[s2]:CH],
                                func=Exp, scale=0.125 / 16)
                    else:
                        nc.scalar.activation(
                            out=pts_t[:], in_=sg_t[:], func=Exp, scale=0.125 / 16)
                    # causal triangle -> 0 on the fp16 weights (post-exp)
                    for s2 in range(2):
                        off = (2 * g + s2) * 128 - i_lo
                        if off >= 0:
                            region = pts_t[:, s2, off:off + 128]
                            nc.gpsimd.affine_select(
                                out=region, in_=region,
                                compare_op=mybir.AluOpType.is_ge,
                                fill=0.0, base=0,
                                pattern=[[1, 128]], channel_multiplier=-1,
                            )
                    pend[(h, g)] = (pts_t, offs)
                    # drain the AV pipeline toward the end of the last chunk
                    # so the tail isn't a serial run of deferred AVs
                    lag = av_lag
                    if c == nch - 1 and idx >= n_groups - 4:
                        lag = 2
                    if idx >= lag:
                        for j in range(idx - av_lag, idx - lag + 1):
                            if j >= 0 and order[j] in pend:
                                emit_av(*order[j])
                    pull((idx + 1) / n_groups)
                for k in range(len(order)):
                    if order[k] in pend:
                        emit_av(*order[k])
                pull(1.0)

            drain(oproj_steps(nch - 1, ys_eng="alt"))

    nc.finalize()
    return nc


def make_in_maps(in_features, q_proj, k_proj, v_proj, o_proj, token_positions, seq=SEQ):
    """Host-side sharding: core c = 4*b + g handles batch b, heads [4g, 4g+4)."""
    f16 = np.float16

    x = np.asarray(in_features, np.float32)
    pos = np.asarray(token_positions, np.float64)
    half = HEAD_DIM // 2
    inv = 1.0 / (THETA ** (2.0 * np.arange(half) / HEAD_DIM))
    ang = pos[:, None] * inv[None, :]           # [seq, 32]
    cosT = np.cos(ang).T                        # [32, seq]
    sinT = np.sin(ang).T
    cosI = np.ascontiguousarray(np.tile(cosT, (4, 1))).astype(f16)
    sinI = np.ascontiguousarray(np.tile(sinT, (4, 1))).astype(f16)

    qp = np.asarray(q_proj, np.float32)
    kp = np.asarray(k_proj, np.float32)
    vp = np.asarray(v_proj, np.float32)
    op = np.asarray(o_proj, np.float32)

    xTb = [np.ascontiguousarray(x[b].T).astype(f16) for b in range(x.shape[0])]
    in_maps = []
    for core in range(N_CORES):
        b, g = divmod(core, HEADS_PER_CORE)
        # row permutation: evens block then odds block, 4 heads x 32
        pe = np.concatenate([64 * (4 * g + h) + np.arange(0, HEAD_DIM, 2)
                             for h in range(HEADS_PER_CORE)])
        po = np.concatenate([64 * (4 * g + h) + np.arange(1, HEAD_DIM, 2)
                             for h in range(HEADS_PER_CORE)])
        perm = np.concatenate([pe, po])
        ks = slice(g * KSLICE, (g + 1) * KSLICE)
        in_maps.append({
            "xT": xTb[b],
            "wq": np.ascontiguousarray(qp[perm].T * 4.0).astype(f16),
            "wk": np.ascontiguousarray(kp[perm].T * 4.0).astype(f16),
            "wv": np.ascontiguousarray(vp[ks].T).astype(f16),
            "wo": np.ascontiguousarray(op[:, ks].T).astype(f16),
            "cosI": cosI,
            "sinI": sinI,
        })
    return in_maps


def assemble(results, batch=2):
    ys = []
    for b in range(batch):
        parts = [results[b * HEADS_PER_CORE + g]["y"].astype(np.float64)
                 for g in range(HEADS_PER_CORE)]
        ys.append(np.sum(parts, axis=0, dtype=np.float64).astype(np.float32))
    return np.stack(ys, axis=0)


_NC_CACHE = {}


def get_nc(seq=SEQ):
    if seq not in _NC_CACHE:
        _NC_CACHE[seq] = build_nc(seq)
    return _NC_CACHE[seq]


def kernel(**inputs):
    from concourse.bass_utils import run_bass_kernel_spmd

    nc = get_nc()
    in_maps = make_in_maps(**inputs)
    res = run_bass_kernel_spmd(nc, in_maps, list(range(N_CORES)))
    return assemble(res.results)


if __name__ == "__main__":
    rng = np.random.default_rng(0)
    ins = {
        "in_features": rng.standard_normal((2, SEQ, D_MODEL), np.float32),
        "q_proj": (rng.standard_normal((D_MODEL, D_MODEL)) * 0.02).astype(np.float32),
        "k_proj": (rng.standard_normal((D_MODEL, D_MODEL)) * 0.02).astype(np.float32),
        "v_proj": (rng.standard_normal((D_MODEL, D_MODEL)) * 0.02).astype(np.float32),
        "o_proj": (rng.standard_normal((D_MODEL, D_MODEL)) * 0.02).astype(np.float32),
        "token_positions": np.arange(SEQ, dtype=np.int32),
    }
    out = kernel(**ins)
    print("kernel output:", out.shape, out.dtype)
